# revision 70
# baseline (speedup 1.0000x reference)
"""Trainium2 Bass kernel v19: WOQ Linear -> +add1+add2 -> WOQ Linear -> mul.

v19 = v18 + partition-major host layouts for all 1KB-row streams (xt, sc,
av, out): the head of the kernel is DMA-descriptor-rate bound, so xt/sc/av
move as [128 x wide] images with 2-8KB contiguous rows (2-4x fewer
descriptors) and k-tile blocks are addressed in PAIR-SLOT order.

Carried from v15-v18 (trace-driven):
 - Layer-1 rank-33 correction (c^T @ r1) computed on HOST and folded into
   a12t: no layer-1 c_mm, no r1 stream.
 - qweight pre-permuted into two contiguous pair-ordered copies (qw1/qw2).
 - Super-1 LAGGED 3 pair-slots behind super 0 over the resident-load
   stream; warmup/filler matmuls on the warm PSUM bank cover the early
   supply deficit and keep the HAM clock gate open.
 - Epilogues deferred past the next super's first chains (engine queues
   are strict FIFO; an epilogue waiting on a PSUM stop must not block the
   dequant stream).  wp bufs=5 lets dequant run ahead at boundaries.
 - Scalar queue carries ONLY the dequant COPY stream mid-kernel; DMA
   issues ride sync (qw, av, out) and gpsimd (xt, sc, consts).  Exception:
   qw pairs 1-3 issue from the scalar ring at the head, where it is
   provably idle (first COPY ~14us in) -- 3-way parallel issue.
 - Layer-2 c_mm hoisted before the final kt-pair; per-bank stop+epilogue;
   drain super pre-casts banks 2-3 on the scalar engine; y1 in bf16.
 - Layer-2 correction matrix + r2 moving operand in bf16; c lives in
   xt_sb's SBUF space (xt is dead after layer 1), converted f32->bf16 by
   the gpsimd Pool DMA in flight.

From v10/v9/v3: kt-pair dequant (2-nibble extract -> contiguous-i16 ACT
cast -> paired mult with a stride-0 broadcast scale), group-interleaved
layer-1 k-tiling (4 scale variants), pi layout making layer-2 gathers
stride-4, in-place qw reload under layer-1's last super, packed rank-33
corrections for layer 2, resident bf16 ar, bf16 streams.
"""

import numpy as np
import ml_dtypes

import concourse.bass as bass  # noqa: F401
from concourse import bacc
import concourse.tile as tile
import concourse.mybir as mybir
from concourse.alu_op_type import AluOpType
from contextlib import ExitStack

BF16 = mybir.dt.bfloat16
F32 = mybir.dt.float32
F32R = mybir.dt.float32r
I32 = mybir.dt.int32
I16 = mybir.dt.int16
BF = ml_dtypes.bfloat16

D = 4096
GS = 128
NPK = 512
G_N = 32
EC = G_N + 1
T_CORE = 512
N_CORES = 8
NSUP = 8
SW = 512

PAIRS = [(a, a + 8) for a in list(range(0, 8)) + list(range(16, 24))]
# pair-slot position of k-tile g (xt / ar blocks are stored in slot order)
SLOT = np.empty(32, dtype=np.int64)
for _p, (_a, _b) in enumerate(PAIRS):
    SLOT[_a], SLOT[_b] = 2 * _p, 2 * _p + 1


def make_pi(d=D):
    pos = np.arange(d)
    s = pos // SW
    c = pos % SW
    return 2048 * (s % 2) + 8 * (c // 2) + (s // 2) + 4 * (c % 2)


def k_perm(d=D):
    g1 = np.arange(d) // 128
    p = np.arange(d) % 128
    return 1024 * (g1 % 4) + 8 * p + (g1 // 4)


def l1_qw_rows():
    """Row order of qw1: pair-major, tile-minor; tile g pulls qweight rows
    1024*(g%4) + (g//4) + 8*p (the v10 stride-8 gather, now contiguous)."""
    rows = np.empty(D, dtype=np.int64)
    p = np.arange(128)
    for pidx, (g0, g1) in enumerate(PAIRS):
        for i, g in enumerate((g0, g1)):
            k0 = 1024 * (g % 4) + (g // 4)
            rows[(2 * pidx + i) * 128:(2 * pidx + i + 1) * 128] = k0 + 8 * p
    return rows


def l2_qw_rows():
    """Row order of qw2: pair-major; tile g pulls qweight rows
    n0 + 4*p with n0 = 2048*(sB%2) + 512*bB + sB//2 (v10 stride-4)."""
    rows = np.empty(D, dtype=np.int64)
    p = np.arange(128)
    for pidx, (g0, g1) in enumerate(PAIRS):
        for i, g in enumerate((g0, g1)):
            sB, bB = g // 4, g % 4
            n0 = 2048 * (sB % 2) + 512 * bB + sB // 2
            rows[(2 * pidx + i) * 128:(2 * pidx + i + 1) * 128] = n0 + 4 * p
    return rows


def build_program(t=T_CORE):
    nc = bacc.Bacc()
    qw1_d = nc.dram_tensor("qw1", [D, NPK], I32, kind="ExternalInput")
    qw2_d = nc.dram_tensor("qw2", [D, NPK], I32, kind="ExternalInput")
    xt_d = nc.dram_tensor("xt_bf", [128, 32 * t], BF16, kind="ExternalInput")
    s1_d = nc.dram_tensor("s1b", [NSUP * 128, 4 * SW], BF16,
                          kind="ExternalInput")
    s2_d = nc.dram_tensor("s2b", [NSUP * 128, 8 * SW], BF16,
                          kind="ExternalInput")
    c_d = nc.dram_tensor("c_mat", [EC, D], F32, kind="ExternalInput")
    r2_d = nc.dram_tensor("r2b", [EC * 2, t], BF16, kind="ExternalInput")
    a12_d = nc.dram_tensor("a12t", [NSUP * 128, 4 * t], BF16,
                           kind="ExternalInput")
    a1_d = nc.dram_tensor("a1t", [NSUP * 128, 4 * t], BF16,
                          kind="ExternalInput")
    out_d = nc.dram_tensor("outt", [128, 32 * t], BF16, kind="ExternalOutput")

    with tile.TileContext(nc) as tc, ExitStack() as ctx:
        const = ctx.enter_context(tc.tile_pool(name="const", bufs=1))
        resid = ctx.enter_context(tc.tile_pool(name="resid", bufs=1))
        scp = ctx.enter_context(tc.tile_pool(name="scp", bufs=3))
        sc8p = ctx.enter_context(tc.tile_pool(name="sc8p", bufs=2))
        nibp = ctx.enter_context(tc.tile_pool(name="nibp", bufs=3))
        nbfp = ctx.enter_context(tc.tile_pool(name="nbfp", bufs=3))
        wp = ctx.enter_context(tc.tile_pool(name="wp", bufs=5))
        avp = ctx.enter_context(tc.tile_pool(name="avp", bufs=2))
        yp = ctx.enter_context(tc.tile_pool(name="yp", bufs=7))
        outp = ctx.enter_context(tc.tile_pool(name="outp", bufs=2))
        otdp = ctx.enter_context(tc.tile_pool(name="otdp", bufs=4))
        psp = ctx.enter_context(tc.tile_pool(name="psp", bufs=8, space="PSUM"))

        r2s = const.tile([97, t], BF16)
        wu = const.tile([128, SW], BF16)

        xt_sb = resid.tile([128, 32 * t], BF16)
        ar_b = resid.tile([128, 32 * t], BF16)
        qw_res = resid.tile([128, 32 * NPK], I32)
        qw_v = qw_res[:].rearrange("p (G c) -> p G c", c=NPK)
        xt_v = xt_sb[:].rearrange("p (G c) -> p G c", c=t)
        ar_v = ar_b[:].rearrange("p (G c) -> p G c", c=t)

        def c_ap(r0, r1, c0, c1):
            # layer-2 correction matrix lives in xt_sb's space (xt is dead
            # after layer 1), in bf16: the gpsimd (Pool) DMA converts the
            # f32 source on the fly
            return xt_sb[r0:r1, c0:c1]

        # PE warmup + filler: open the HAM clock gate and keep the PE busy
        # until the first kt-pair lands; ps_warm's bank is reused by
        # super-1's lagged accumulation which only starts at slot 3.
        nc.vector.memset(wu[:], 0.0)
        ps_warm = psp.tile([128, t], F32, tag="ps", name="ps_warm")
        for _ in range(32):
            nc.tensor.matmul(ps_warm[:], wu[:, 0:128], wu[:],
                             start=True, stop=True)

        def sc_tile(layer, s):
            nv = 4 if layer == 1 else 8
            pool = scp if layer == 1 else sc8p
            return pool.tile([128, nv, SW], BF16, tag=f"sc{nv}",
                             name=f"sc_{layer}_{s}")

        def load_sc1(sc, s, v, eng=None):
            # per-variant 2D slice out of the partition-major scale image
            # (head path: fine-grained, spread across rings)
            (eng or nc.gpsimd).dma_start(
                sc[:, v, :], s1_d[s * 128:(s + 1) * 128, v * SW:(v + 1) * SW])
            return sc[:, v, :]

        def load_sc_all(layer, s, eng=None):
            # one dma_start per super: [128 x nv*SW] with 4-8KB rows
            nv = 4 if layer == 1 else 8
            sc_d = s1_d if layer == 1 else s2_d
            sc = sc_tile(layer, s)
            (eng or nc.gpsimd).dma_start(
                sc[:], sc_d[s * 128:(s + 1) * 128, :].rearrange(
                    "p (v c) -> p v c", c=SW))
            return [sc[:, v, :] for v in range(nv)]

        def chain(layer, s, pidx, scs, ps, rhs_v, stop_last=False):
            """dequant chain + 8 matmuls for (super s, kt-pair pidx)."""
            jj, hh = s // 2, s % 2
            g0, g1 = PAIRS[pidx]
            qs = qw_v[:, g0:g0 + 9:8, 256 * hh:256 * hh + 256]
            nib = nibp.tile([128, SW], I32, tag="nib",
                            name=f"nib_{layer}_{s}_{pidx}")
            nc.vector.tensor_scalar(
                nib[:].rearrange("p (a c) -> p a c", a=2), qs,
                4 * jj, 0x000F000F,
                AluOpType.logical_shift_right, AluOpType.bitwise_and)
            nbf = nbfp.tile([128, 2 * SW], BF16, tag="nbf",
                            name=f"nbf_{layer}_{s}_{pidx}")
            nc.scalar.copy(nbf[:], nib[:].bitcast(I16))
            w_t = wp.tile([128, 2 * SW], BF16, tag="w",
                          name=f"w_{layer}_{s}_{pidx}")
            v = (g0 % 4) if layer == 1 else 4 * ((g0 // 4) % 2) + (g0 % 4)
            nc.vector.tensor_tensor(
                w_t[:].rearrange("p (i c) -> p i c", i=2),
                nbf[:].rearrange("p (i c) -> p i c", i=2),
                scs[v].unsqueeze(1).broadcast_to([128, 2, SW]),
                AluOpType.mult)
            for i in range(2):
                rhs = rhs_v[:, 2 * pidx + i, :]
                for b in range(4):
                    nc.tensor.matmul(
                        ps[b][:], w_t[:, i * SW + b * 128:i * SW + (b + 1) * 128],
                        rhs, start=(pidx == 0 and i == 0),
                        stop=(stop_last and i == 1))

        def c_mm(s, b, ps, r_sb):
            p0 = 64 * (b % 2)
            c0 = s * SW + b * 128
            nc.tensor.matmul(
                ps[b][:], c_ap(p0, p0 + EC, c0, c0 + 128),
                r_sb[p0:p0 + EC, :], start=False, stop=False,
                tile_position=(p0, 0))

        def load_av(layer, s, eng=None, half=None):
            # one (or two half) dma_starts per super: 2-4KB rows
            av_d = a12_d if layer == 1 else a1_d
            av = avp.tile([128, 4, t], BF16, tag="av", name=f"av_{layer}_{s}")
            lo, hi = (0, 4) if half is None else ((0, 2) if half == 0 else (2, 4))
            (eng or nc.sync).dma_start(
                av[:, lo:hi, :],
                av_d[s * 128:(s + 1) * 128, lo * t:hi * t].rearrange(
                    "p (b c) -> p b c", c=t))
            return av

        def load_av_half(av, layer, s, eng=None):
            av_d = a12_d if layer == 1 else a1_d
            (eng or nc.sync).dma_start(
                av[:, 2:4, :],
                av_d[s * 128:(s + 1) * 128, 2 * t:4 * t].rearrange(
                    "p (b c) -> p b c", c=t))

        def epilogue1(s, b, ps, av):
            g2 = 4 * s + b
            sl = SLOT[g2]
            nc.vector.tensor_tensor(ar_b[:, sl * t:(sl + 1) * t],
                                    ps[b][:], av[:, b, :], AluOpType.add)

        def epilogue2(s, b, ps, av, ot, oti, pre=None):
            g2 = 4 * s + b
            sl = SLOT[g2]
            y1 = yp.tile([128, t], BF16, tag="y", name=f"y_{s}_{b}")
            nc.vector.tensor_tensor(y1[:], (pre or ps[b])[:], av[:, b, :],
                                    AluOpType.add)
            nc.vector.tensor_tensor(ot[:, oti, :], y1[:],
                                    ar_b[:, sl * t:(sl + 1) * t],
                                    AluOpType.mult)

        def load_pair(qd, pidx, eng=None):
            g0, g1 = PAIRS[pidx]
            for i, g in enumerate((g0, g1)):
                r0 = (2 * pidx + i) * 128
                (eng or nc.sync).dma_start(qw_v[:, g, :], qd[r0:r0 + 128, :])

        # ================= layer 1 =================
        # Supers 0 and 1 ride the resident-load stream, with super 1 LAGGED
        # by 3 pair-slots: during slots 0-2 only super-0's 8 MMs consume a
        # fresh pair, and filler matmuls on the warm bank cover the early
        # DMA-supply deficit -- also keeping the HAM clock gate open.
        # Super 1 finishes at slots 16-18, overlapping super 2.
        LAG = 3
        FILL = {0: 12, 1: 8, 2: 6}
        sc0_t = sc_tile(1, 0)
        sc1_t = sc_tile(1, 1)
        scs0 = []
        scs1 = []
        sc_l1 = {0: scs0, 1: scs1}
        av0 = av1 = None
        ps0 = [psp.tile([128, t], F32, tag="ps", name=f"ps_1_0_{b}")
               for b in range(4)]
        ps1 = None
        for slot in range(16 + LAG):
            if slot < 16:
                pidx = slot
                # pairs 1-3 issue from the scalar ring: it is idle until its
                # first dequant COPY (~14us), so the head gets 3-way issue
                load_pair(qw1_d, pidx,
                          eng=nc.scalar if pidx in (1, 2, 3) else None)
                nc.gpsimd.dma_start(
                    xt_v[:, 2 * pidx:2 * pidx + 2, :],
                    xt_d[:, 2 * pidx * t:(2 * pidx + 2) * t].rearrange(
                        "p (i c) -> p i c", c=t))
                if pidx == 0:
                    scs0.append(load_sc1(sc0_t, 0, 0, eng=nc.sync))
                    scs1.append(load_sc1(sc1_t, 1, 0, eng=nc.sync))
                if pidx in (0, 1, 2):
                    # stagger remaining variants on gpsimd, a slot ahead
                    scs0.append(load_sc1(sc0_t, 0, pidx + 1))
                    scs1.append(load_sc1(sc1_t, 1, pidx + 1))
                if pidx == 11:
                    av0 = load_av(1, 0, eng=nc.sync, half=0)
                if pidx == 12:
                    load_av_half(av0, 1, 0, eng=nc.sync)
                if pidx == 13:
                    av1 = load_av(1, 1, eng=nc.sync, half=0)
                if pidx == 14:
                    load_av_half(av1, 1, 1, eng=nc.sync)
                if pidx == 15:
                    sc_l1[2] = load_sc_all(1, 2, eng=nc.sync)
                if slot < LAG:
                    # filler BEFORE the chain: it must bridge the window
                    # between the upfront warmup and this pair's arrival
                    # (the PE queue is FIFO)
                    for _ in range(FILL[slot]):
                        nc.tensor.matmul(ps_warm[:], wu[:, 0:128], wu[:],
                                         start=True, stop=True)
                chain(1, 0, pidx, scs0, ps0, xt_v, stop_last=(pidx == 15))
            if slot >= LAG:
                if ps1 is None:
                    ps1 = [psp.tile([128, t], F32, tag="ps",
                                    name=f"ps_1_1_{b}") for b in range(4)]
                chain(1, 1, slot - LAG, scs1, ps1, xt_v,
                      stop_last=(slot - LAG == 15))
            if slot == 17:
                # super-0 stops landed two slots ago; its epilogue here does
                # not stall the vector queue and frees ps0 for super 2
                for b in range(4):
                    epilogue1(0, b, ps0, av0)

        # supers 2..7 with 4+4 psum ping-pong; scales prefetched mid-super;
        # each super's epilogue is emitted after the NEXT super's first
        # chains so the (strict-FIFO) vector queue never stalls on a
        # PSUM-stop wait between supers
        pend1 = (1, ps1, av1)
        for s in range(2, NSUP):
            scs = sc_l1[s]
            av = load_av(1, s)
            ps = [psp.tile([128, t], F32, tag="ps", name=f"ps_1_{s}_{b}")
                  for b in range(4)]
            for pidx in range(16):
                if pidx == 8 and s + 1 < NSUP:
                    sc_l1[s + 1] = load_sc_all(1, s + 1)
                if pidx == 2 and s == 3:
                    # host-precomputed r2 rows (0:33 and duplicated 64:97)
                    nc.gpsimd.dma_start(r2s[0:EC, :], r2_d[0:EC, :])
                    nc.gpsimd.dma_start(r2s[64:64 + EC, :], r2_d[EC:2 * EC, :])
                chain(1, s, pidx, scs, ps, xt_v, stop_last=(pidx == 15))
                if pidx == 1:
                    ls, lps, lav = pend1
                    for b in range(4):
                        epilogue1(ls, b, lps, lav)
            pend1 = (s, ps, av)

        # qw reload for layer 2 (in-place; WAR-gated on super-7 reads)
        for pidx in range(16):
            load_pair(qw2_d, pidx)

        # layer-2 correction matrix into xt_sb's space (xt now dead)
        nc.gpsimd.dma_start(c_ap(0, EC, 0, D), c_d[:])
        nc.gpsimd.dma_start(c_ap(64, 64 + EC, 0, D), c_d[:])

        # super-7's epilogue: the e2 matmuls for its ar tiles depend on it
        ls, lps, lav = pend1
        for b in range(4):
            epilogue1(ls, b, lps, lav)

        # ================= layer 2 =================
        # epilogues deferred past the next super's first chains; output
        # written via 2-bank [128 x 1024] tiles (2KB DRAM rows); last super
        # drains inline with per-bank tiles + scalar pre-cast of banks 2-3
        sc_l2 = {0: load_sc_all(2, 0)}
        pend2 = None
        for s in range(NSUP):
            scs = sc_l2[s]
            av = load_av(2, s)
            ps = [psp.tile([128, t], F32, tag="ps", name=f"ps_2_{s}_{b}")
                  for b in range(4)]
            for pidx in range(15):
                if pidx == 8 and s + 1 < NSUP:
                    sc_l2[s + 1] = load_sc_all(2, s + 1)
                chain(2, s, pidx, scs, ps, ar_v)
                if pidx == 1 and pend2 is not None:
                    ls, lps, lav = pend2
                    for half in range(2):
                        ot = outp.tile([128, 2, t], BF16, tag="ot",
                                       name=f"ot_{ls}_{half}")
                        for oti in range(2):
                            epilogue2(ls, 2 * half + oti, lps, lav, ot, oti)
                        c0 = (4 * ls + 2 * half) * t
                        nc.sync.dma_start(
                            out_d[:, c0:c0 + 2 * t].rearrange(
                                "p (i c) -> p i c", c=t), ot[:])
                    pend2 = None
            # corrections before the final pair: PSUM accumulation is
            # order-independent, so the tail drains without extra matmuls
            for b in range(4):
                c_mm(s, b, ps, r2s)
            chain(2, s, 15, scs, ps, ar_v, stop_last=True)
            if s < NSUP - 1:
                pend2 = (s, ps, av)
            else:
                # drain: scalar pre-casts banks 1-3 off the vector engine;
                # final out DMAs alternate sync/gpsimd so their ~0.6us
                # issues don't serialize behind the last MULs
                pre = {}
                for b in (1, 2, 3):
                    pre[b] = yp.tile([128, t], BF16, tag="y", name=f"y2b_{b}")
                    nc.scalar.copy(pre[b][:], ps[b][:])
                for b in range(4):
                    ot = otdp.tile([128, 1, t], BF16, tag="otd",
                                   name=f"otd_{b}")
                    epilogue2(s, b, ps, av, ot, 0, pre=pre.get(b))
                    c0 = (4 * s + b) * t
                    eng = nc.sync if b % 2 == 0 else nc.gpsimd
                    eng.dma_start(out_d[:, c0:c0 + t], ot[:, 0, :])
    nc.compile()
    return nc


def host_prep(inp, qweight, woq_scales, woq_qzeros, woq_bias, add1, add2,
              t=T_CORE, n_cores=N_CORES):
    pi = make_pi()
    kp = k_perm()
    rows1 = l1_qw_rows()
    rows2 = l2_qw_rows()
    x = inp.reshape(-1, D)
    a1 = add1.reshape(-1, D)
    a12 = (a1 + add2.reshape(-1, D))

    shifts = (np.arange(8, dtype=np.int32) * 4)
    z = ((woq_qzeros[:, :, None] >> shifts) & 0xF).reshape(G_N, D).astype(np.float32)
    zs = z * woq_scales
    c_mat = np.empty((EC, D), dtype=np.float32)
    c_mat[:G_N] = -zs[:, pi]
    c_mat[G_N] = woq_bias[pi]

    s_bf = woq_scales.astype(BF)
    pi_cols = pi.reshape(NSUP, SW)
    g1_row = 8 * np.arange(4)[:, None] + np.arange(128)[None, :] // 16
    # [s, v, p, c] -> partition-major [s, p, v, c] -> [NSUP*128, 4*SW]
    s1b = s_bf[g1_row[None, :, :, None], pi_cols[:, None, None, :]]
    s1b = np.ascontiguousarray(
        s1b.transpose(0, 2, 1, 3).reshape(NSUP * 128, 4 * SW))
    hbi = np.arange(8)
    G0 = 16 * (hbi // 4) + 4 * (hbi % 4)
    g2_row = G0[:, None] + np.arange(128)[None, :] // 32
    s2b = s_bf[g2_row[None, :, :, None], pi_cols[:, None, None, :]]
    s2b = np.ascontiguousarray(
        s2b.transpose(0, 2, 1, 3).reshape(NSUP * 128, 8 * SW))
    # host-side r2: group-summed dequantized weights (device-faithful bf16
    # rounding per element; sum-before-ar-rounding approximation is ~0.4%
    # of the correction term)
    qn = ((qweight[:, :, None] >> shifts) & 0xF).reshape(D, D)
    s_exp = s_bf.astype(np.float32)[np.arange(D) // GS]
    wb = (qn * s_exp).astype(BF).astype(np.float32)
    jpos = np.arange(D)
    g2j = jpos // 128
    hbj = 4 * ((g2j // 4) % 2) + (g2j % 4)
    ej = 16 * (hbj // 4) + 4 * (hbj % 4) + (jpos % 128) // 32
    onehot = np.zeros((D, G_N), dtype=np.float32)
    onehot[jpos, ej] = 1
    ws = wb[:, pi] @ onehot  # [D(k), 32]

    qw1 = np.ascontiguousarray(qweight[rows1])
    qw2 = np.ascontiguousarray(qweight[rows2])

    def pm(arr):
        # [D(=32 blocks of 128), t] row-major -> partition-major
        # [128, 32*t] with blocks along columns
        return np.ascontiguousarray(
            arr.reshape(32, 128, t).transpose(1, 0, 2).reshape(128, 32 * t))

    def pm_av(arr):
        # [D, t] -> per-super partition-major [NSUP*128, 4*t]
        return np.ascontiguousarray(
            arr.reshape(NSUP, 4, 128, t).transpose(0, 2, 1, 3).reshape(
                NSUP * 128, 4 * t))

    in_maps = []
    for i in range(n_cores):
        sl = slice(i * t, (i + 1) * t)
        xtb_nat = np.ascontiguousarray(x[sl].T).astype(BF)
        r1 = np.ones((EC, t), dtype=np.float32)
        r1[:G_N] = xtb_nat.astype(np.float32).reshape(G_N, GS, t).sum(axis=1)
        corr = c_mat.T @ r1  # [D(pi-order), t] layer-1 correction, exact
        a12t = (a12[sl][:, pi].T + corr).astype(BF)
        r2 = (xtb_nat.astype(np.float32).T @ ws).T \
            + onehot.T @ a12t.astype(np.float32)  # [32, t]
        r2b = np.ones((EC * 2, t), dtype=np.float32)
        r2b[0:G_N] = r2
        r2b[EC:EC + G_N] = r2
        xts = xtb_nat[kp].reshape(32, 128, t)[SLOT_INV]  # slot-major tiles
        in_maps.append({
            "qw1": qw1,
            "qw2": qw2,
            "xt_bf": np.ascontiguousarray(
                xts.transpose(1, 0, 2).reshape(128, 32 * t)),
            "s1b": s1b,
            "s2b": s2b,
            "c_mat": c_mat,
            "r2b": r2b.astype(BF),
            "a12t": pm_av(a12t),
            "a1t": pm_av(np.ascontiguousarray(a1[sl][:, pi].T).astype(BF)),
        })
    return in_maps, pi


# tile that sits at slot j of the pair-major order
SLOT_INV = np.argsort(SLOT)

_CACHE = {}


def kernel(inp, qweight, woq_scales, woq_qzeros, woq_bias, add1, add2,
           group_size=GS, _trace=False, _repeat=1):
    from concourse import bass_utils
    inp = np.asarray(inp, dtype=np.float32)
    qweight = np.asarray(qweight, dtype=np.int32)
    woq_scales = np.asarray(woq_scales, dtype=np.float32)
    woq_qzeros = np.asarray(woq_qzeros, dtype=np.int32)
    woq_bias = np.asarray(woq_bias, dtype=np.float32)
    add1 = np.asarray(add1, dtype=np.float32)
    add2 = np.asarray(add2, dtype=np.float32)

    if "nc" not in _CACHE:
        _CACHE["nc"] = build_program()
    nc = _CACHE["nc"]
    in_maps, pi = host_prep(inp, qweight, woq_scales, woq_qzeros, woq_bias,
                            add1, add2)
    import time as _time
    times = []
    res = None
    for _ in range(max(1, _repeat)):
        t0 = _time.time()
        res = bass_utils.run_bass_kernel_spmd(
            nc, in_maps, list(range(N_CORES)), trace=_trace)
        times.append(_time.time() - t0)
    _CACHE["times"] = times
    out = np.empty((N_CORES * T_CORE, D), dtype=np.float32)
    for i in range(N_CORES):
        outt = res.results[i]["outt"].astype(np.float32)
        # [128, 32*t] partition-major, blocks in g2 order -> [t, D(pi)]
        feat = outt.reshape(128, 32, T_CORE).transpose(1, 0, 2).reshape(
            D, T_CORE)
        out[i * T_CORE:(i + 1) * T_CORE][:, pi] = feat.T
    _CACHE["last_result"] = res
    return out.reshape(inp.shape[0], inp.shape[1], D)


# revision 71
# speedup vs baseline: 1.0043x; 1.0043x over previous
"""Trainium2 Bass kernel v19: WOQ Linear -> +add1+add2 -> WOQ Linear -> mul.

v19 = v18 + partition-major host layouts for all 1KB-row streams (xt, sc,
av, out): the head of the kernel is DMA-descriptor-rate bound, so xt/sc/av
move as [128 x wide] images with 2-8KB contiguous rows (2-4x fewer
descriptors) and k-tile blocks are addressed in PAIR-SLOT order.

Carried from v15-v18 (trace-driven):
 - Layer-1 rank-33 correction (c^T @ r1) computed on HOST and folded into
   a12t: no layer-1 c_mm, no r1 stream.
 - qweight pre-permuted into two contiguous pair-ordered copies (qw1/qw2).
 - Super-1 LAGGED 3 pair-slots behind super 0 over the resident-load
   stream; warmup/filler matmuls on the warm PSUM bank cover the early
   supply deficit and keep the HAM clock gate open.
 - Epilogues deferred past the next super's first chains (engine queues
   are strict FIFO; an epilogue waiting on a PSUM stop must not block the
   dequant stream).  wp bufs=5 lets dequant run ahead at boundaries.
 - Scalar queue carries ONLY the dequant COPY stream mid-kernel; DMA
   issues ride sync (qw, av, out) and gpsimd (xt, sc, consts).  Exception:
   qw pairs 1-3 issue from the scalar ring at the head, where it is
   provably idle (first COPY ~14us in) -- 3-way parallel issue.
 - Layer-2 c_mm hoisted before the final kt-pair; per-bank stop+epilogue;
   drain super pre-casts banks 2-3 on the scalar engine; y1 in bf16.
 - Layer-2 correction matrix + r2 moving operand in bf16; c lives in
   xt_sb's SBUF space (xt is dead after layer 1), converted f32->bf16 by
   the gpsimd Pool DMA in flight.

From v10/v9/v3: kt-pair dequant (2-nibble extract -> contiguous-i16 ACT
cast -> paired mult with a stride-0 broadcast scale), group-interleaved
layer-1 k-tiling (4 scale variants), pi layout making layer-2 gathers
stride-4, in-place qw reload under layer-1's last super, packed rank-33
corrections for layer 2, resident bf16 ar, bf16 streams.
"""

import numpy as np
import ml_dtypes

import concourse.bass as bass  # noqa: F401
from concourse import bacc
import concourse.tile as tile
import concourse.mybir as mybir
from concourse.alu_op_type import AluOpType
from contextlib import ExitStack

BF16 = mybir.dt.bfloat16
F32 = mybir.dt.float32
F32R = mybir.dt.float32r
I32 = mybir.dt.int32
I16 = mybir.dt.int16
BF = ml_dtypes.bfloat16

D = 4096
GS = 128
NPK = 512
G_N = 32
EC = G_N + 1
T_CORE = 512
N_CORES = 8
NSUP = 8
SW = 512

PAIRS = [(a, a + 8) for a in list(range(0, 8)) + list(range(16, 24))]
# pair-slot position of k-tile g (xt / ar blocks are stored in slot order)
SLOT = np.empty(32, dtype=np.int64)
for _p, (_a, _b) in enumerate(PAIRS):
    SLOT[_a], SLOT[_b] = 2 * _p, 2 * _p + 1


def make_pi(d=D):
    pos = np.arange(d)
    s = pos // SW
    c = pos % SW
    return 2048 * (s % 2) + 8 * (c // 2) + (s // 2) + 4 * (c % 2)


def k_perm(d=D):
    g1 = np.arange(d) // 128
    p = np.arange(d) % 128
    return 1024 * (g1 % 4) + 8 * p + (g1 // 4)


def l1_qw_rows():
    """Row order of qw1: pair-major, tile-minor; tile g pulls qweight rows
    1024*(g%4) + (g//4) + 8*p (the v10 stride-8 gather, now contiguous)."""
    rows = np.empty(D, dtype=np.int64)
    p = np.arange(128)
    for pidx, (g0, g1) in enumerate(PAIRS):
        for i, g in enumerate((g0, g1)):
            k0 = 1024 * (g % 4) + (g // 4)
            rows[(2 * pidx + i) * 128:(2 * pidx + i + 1) * 128] = k0 + 8 * p
    return rows


def l2_qw_rows():
    """Row order of qw2: pair-major; tile g pulls qweight rows
    n0 + 4*p with n0 = 2048*(sB%2) + 512*bB + sB//2 (v10 stride-4)."""
    rows = np.empty(D, dtype=np.int64)
    p = np.arange(128)
    for pidx, (g0, g1) in enumerate(PAIRS):
        for i, g in enumerate((g0, g1)):
            sB, bB = g // 4, g % 4
            n0 = 2048 * (sB % 2) + 512 * bB + sB // 2
            rows[(2 * pidx + i) * 128:(2 * pidx + i + 1) * 128] = n0 + 4 * p
    return rows


def build_program(t=T_CORE):
    nc = bacc.Bacc()
    qw1_d = nc.dram_tensor("qw1", [D, NPK], I32, kind="ExternalInput")
    qw2_d = nc.dram_tensor("qw2", [D, NPK], I32, kind="ExternalInput")
    xt_d = nc.dram_tensor("xt_bf", [128, 32 * t], BF16, kind="ExternalInput")
    s1_d = nc.dram_tensor("s1b", [NSUP * 128, 4 * SW], BF16,
                          kind="ExternalInput")
    s2_d = nc.dram_tensor("s2b", [NSUP * 128, 8 * SW], BF16,
                          kind="ExternalInput")
    c_d = nc.dram_tensor("c_mat", [EC, D], F32, kind="ExternalInput")
    r2_d = nc.dram_tensor("r2b", [EC * 2, t], BF16, kind="ExternalInput")
    a12_d = nc.dram_tensor("a12t", [NSUP * 128, 4 * t], BF16,
                           kind="ExternalInput")
    a1_d = nc.dram_tensor("a1t", [NSUP * 128, 4 * t], BF16,
                          kind="ExternalInput")
    out_d = nc.dram_tensor("outt", [128, 32 * t], BF16, kind="ExternalOutput")

    with tile.TileContext(nc) as tc, ExitStack() as ctx:
        const = ctx.enter_context(tc.tile_pool(name="const", bufs=1))
        resid = ctx.enter_context(tc.tile_pool(name="resid", bufs=1))
        scp = ctx.enter_context(tc.tile_pool(name="scp", bufs=3))
        sc8p = ctx.enter_context(tc.tile_pool(name="sc8p", bufs=2))
        nibp = ctx.enter_context(tc.tile_pool(name="nibp", bufs=3))
        nbfp = ctx.enter_context(tc.tile_pool(name="nbfp", bufs=3))
        wp = ctx.enter_context(tc.tile_pool(name="wp", bufs=5))
        avp = ctx.enter_context(tc.tile_pool(name="avp", bufs=2))
        yp = ctx.enter_context(tc.tile_pool(name="yp", bufs=6))
        outp = ctx.enter_context(tc.tile_pool(name="outp", bufs=2))
        otdp = ctx.enter_context(tc.tile_pool(name="otdp", bufs=4))
        psp = ctx.enter_context(tc.tile_pool(name="psp", bufs=8, space="PSUM"))

        r2s = const.tile([97, t], BF16)
        wu = const.tile([128, SW], BF16)

        xt_sb = resid.tile([128, 32 * t], BF16)
        ar_b = resid.tile([128, 32 * t], BF16)
        qw_res = resid.tile([128, 32 * NPK], I32)
        qw_v = qw_res[:].rearrange("p (G c) -> p G c", c=NPK)
        xt_v = xt_sb[:].rearrange("p (G c) -> p G c", c=t)
        ar_v = ar_b[:].rearrange("p (G c) -> p G c", c=t)

        def c_ap(r0, r1, c0, c1):
            # layer-2 correction matrix lives in xt_sb's space (xt is dead
            # after layer 1), in bf16: the gpsimd (Pool) DMA converts the
            # f32 source on the fly
            return xt_sb[r0:r1, c0:c1]

        # PE warmup + filler: open the HAM clock gate and keep the PE busy
        # until the first kt-pair lands; ps_warm's bank is reused by
        # super-1's lagged accumulation which only starts at slot 3.
        nc.vector.memset(wu[:], 0.0)
        ps_warm = psp.tile([128, t], F32, tag="ps", name="ps_warm")
        for _ in range(32):
            nc.tensor.matmul(ps_warm[:], wu[:, 0:128], wu[:],
                             start=True, stop=True)

        def sc_tile(layer, s):
            nv = 4 if layer == 1 else 8
            pool = scp if layer == 1 else sc8p
            return pool.tile([128, nv, SW], BF16, tag=f"sc{nv}",
                             name=f"sc_{layer}_{s}")

        def load_sc1(sc, s, v, eng=None):
            # per-variant 2D slice out of the partition-major scale image
            # (head path: fine-grained, spread across rings)
            (eng or nc.gpsimd).dma_start(
                sc[:, v, :], s1_d[s * 128:(s + 1) * 128, v * SW:(v + 1) * SW])
            return sc[:, v, :]

        def load_sc_all(layer, s, eng=None):
            # one dma_start per super: [128 x nv*SW] with 4-8KB rows
            nv = 4 if layer == 1 else 8
            sc_d = s1_d if layer == 1 else s2_d
            sc = sc_tile(layer, s)
            (eng or nc.gpsimd).dma_start(
                sc[:], sc_d[s * 128:(s + 1) * 128, :].rearrange(
                    "p (v c) -> p v c", c=SW))
            return [sc[:, v, :] for v in range(nv)]

        def chain(layer, s, pidx, scs, ps, rhs_v, stop_last=False):
            """dequant chain + 8 matmuls for (super s, kt-pair pidx)."""
            jj, hh = s // 2, s % 2
            g0, g1 = PAIRS[pidx]
            qs = qw_v[:, g0:g0 + 9:8, 256 * hh:256 * hh + 256]
            nib = nibp.tile([128, SW], I32, tag="nib",
                            name=f"nib_{layer}_{s}_{pidx}")
            nc.vector.tensor_scalar(
                nib[:].rearrange("p (a c) -> p a c", a=2), qs,
                4 * jj, 0x000F000F,
                AluOpType.logical_shift_right, AluOpType.bitwise_and)
            nbf = nbfp.tile([128, 2 * SW], BF16, tag="nbf",
                            name=f"nbf_{layer}_{s}_{pidx}")
            nc.scalar.copy(nbf[:], nib[:].bitcast(I16))
            w_t = wp.tile([128, 2 * SW], BF16, tag="w",
                          name=f"w_{layer}_{s}_{pidx}")
            v = (g0 % 4) if layer == 1 else 4 * ((g0 // 4) % 2) + (g0 % 4)
            nc.vector.tensor_tensor(
                w_t[:].rearrange("p (i c) -> p i c", i=2),
                nbf[:].rearrange("p (i c) -> p i c", i=2),
                scs[v].unsqueeze(1).broadcast_to([128, 2, SW]),
                AluOpType.mult)
            for i in range(2):
                rhs = rhs_v[:, 2 * pidx + i, :]
                for b in range(4):
                    nc.tensor.matmul(
                        ps[b][:], w_t[:, i * SW + b * 128:i * SW + (b + 1) * 128],
                        rhs, start=(pidx == 0 and i == 0),
                        stop=(stop_last and i == 1))

        def c_mm(s, b, ps, r_sb):
            p0 = 64 * (b % 2)
            c0 = s * SW + b * 128
            nc.tensor.matmul(
                ps[b][:], c_ap(p0, p0 + EC, c0, c0 + 128),
                r_sb[p0:p0 + EC, :], start=False, stop=False,
                tile_position=(p0, 0))

        def load_av(layer, s, eng=None, half=None):
            # one (or two half) dma_starts per super: 2-4KB rows
            av_d = a12_d if layer == 1 else a1_d
            av = avp.tile([128, 4, t], BF16, tag="av", name=f"av_{layer}_{s}")
            lo, hi = (0, 4) if half is None else ((0, 2) if half == 0 else (2, 4))
            (eng or nc.sync).dma_start(
                av[:, lo:hi, :],
                av_d[s * 128:(s + 1) * 128, lo * t:hi * t].rearrange(
                    "p (b c) -> p b c", c=t))
            return av

        def load_av_half(av, layer, s, eng=None):
            av_d = a12_d if layer == 1 else a1_d
            (eng or nc.sync).dma_start(
                av[:, 2:4, :],
                av_d[s * 128:(s + 1) * 128, 2 * t:4 * t].rearrange(
                    "p (b c) -> p b c", c=t))

        def epilogue1(s, b, ps, av):
            g2 = 4 * s + b
            sl = SLOT[g2]
            nc.vector.tensor_tensor(ar_b[:, sl * t:(sl + 1) * t],
                                    ps[b][:], av[:, b, :], AluOpType.add)

        def epilogue2(s, b, ps, av, ot, oti, pre=None):
            g2 = 4 * s + b
            sl = SLOT[g2]
            y1 = yp.tile([128, t], BF16, tag="y", name=f"y_{s}_{b}")
            nc.vector.tensor_tensor(y1[:], (pre or ps[b])[:], av[:, b, :],
                                    AluOpType.add)
            nc.vector.tensor_tensor(ot[:, oti, :], y1[:],
                                    ar_b[:, sl * t:(sl + 1) * t],
                                    AluOpType.mult)

        def load_pair(qd, pidx, eng=None):
            g0, g1 = PAIRS[pidx]
            for i, g in enumerate((g0, g1)):
                r0 = (2 * pidx + i) * 128
                (eng or nc.sync).dma_start(qw_v[:, g, :], qd[r0:r0 + 128, :])

        # ================= layer 1 =================
        # Supers 0 and 1 ride the resident-load stream, with super 1 LAGGED
        # by 3 pair-slots: during slots 0-2 only super-0's 8 MMs consume a
        # fresh pair, and filler matmuls on the warm bank cover the early
        # DMA-supply deficit -- also keeping the HAM clock gate open.
        # Super 1 finishes at slots 16-18, overlapping super 2.
        LAG = 3
        FILL = {0: 12, 1: 8, 2: 6}
        sc0_t = sc_tile(1, 0)
        sc1_t = sc_tile(1, 1)
        scs0 = []
        scs1 = []
        sc_l1 = {0: scs0, 1: scs1}
        av0 = av1 = None
        ps0 = [psp.tile([128, t], F32, tag="ps", name=f"ps_1_0_{b}")
               for b in range(4)]
        ps1 = None
        for slot in range(16 + LAG):
            if slot < 16:
                pidx = slot
                # pairs 1-3 issue from the scalar ring: it is idle until its
                # first dequant COPY (~14us), so the head gets 3-way issue
                load_pair(qw1_d, pidx,
                          eng=nc.scalar if pidx in (1, 2, 3) else None)
                nc.gpsimd.dma_start(
                    xt_v[:, 2 * pidx:2 * pidx + 2, :],
                    xt_d[:, 2 * pidx * t:(2 * pidx + 2) * t].rearrange(
                        "p (i c) -> p i c", c=t))
                if pidx == 0:
                    scs0.append(load_sc1(sc0_t, 0, 0, eng=nc.sync))
                    scs1.append(load_sc1(sc1_t, 1, 0, eng=nc.sync))
                if pidx in (0, 1, 2):
                    # stagger remaining variants on gpsimd, a slot ahead
                    scs0.append(load_sc1(sc0_t, 0, pidx + 1))
                    scs1.append(load_sc1(sc1_t, 1, pidx + 1))
                if pidx == 11:
                    av0 = load_av(1, 0, eng=nc.sync, half=0)
                if pidx == 12:
                    load_av_half(av0, 1, 0, eng=nc.sync)
                if pidx == 13:
                    av1 = load_av(1, 1, eng=nc.sync, half=0)
                if pidx == 14:
                    load_av_half(av1, 1, 1, eng=nc.sync)
                if pidx == 15:
                    sc_l1[2] = load_sc_all(1, 2, eng=nc.sync)
                if slot < LAG:
                    # filler BEFORE the chain: it must bridge the window
                    # between the upfront warmup and this pair's arrival
                    # (the PE queue is FIFO)
                    for _ in range(FILL[slot]):
                        nc.tensor.matmul(ps_warm[:], wu[:, 0:128], wu[:],
                                         start=True, stop=True)
                chain(1, 0, pidx, scs0, ps0, xt_v, stop_last=(pidx == 15))
            if slot >= LAG:
                if ps1 is None:
                    ps1 = [psp.tile([128, t], F32, tag="ps",
                                    name=f"ps_1_1_{b}") for b in range(4)]
                chain(1, 1, slot - LAG, scs1, ps1, xt_v,
                      stop_last=(slot - LAG == 15))
            if slot == 17:
                # super-0 stops landed two slots ago; its epilogue here does
                # not stall the vector queue and frees ps0 for super 2
                for b in range(4):
                    epilogue1(0, b, ps0, av0)

        # supers 2..7 with 4+4 psum ping-pong; scales prefetched mid-super;
        # each super's epilogue is emitted after the NEXT super's first
        # chains so the (strict-FIFO) vector queue never stalls on a
        # PSUM-stop wait between supers
        pend1 = (1, ps1, av1)
        for s in range(2, NSUP):
            scs = sc_l1[s]
            av = load_av(1, s)
            ps = [psp.tile([128, t], F32, tag="ps", name=f"ps_1_{s}_{b}")
                  for b in range(4)]
            for pidx in range(16):
                if pidx == 8 and s + 1 < NSUP:
                    sc_l1[s + 1] = load_sc_all(1, s + 1)
                if pidx == 2 and s == 3:
                    # host-precomputed r2 rows (0:33 and duplicated 64:97)
                    nc.gpsimd.dma_start(r2s[0:EC, :], r2_d[0:EC, :])
                    nc.gpsimd.dma_start(r2s[64:64 + EC, :], r2_d[EC:2 * EC, :])
                chain(1, s, pidx, scs, ps, xt_v, stop_last=(pidx == 15))
                if pidx == 1:
                    ls, lps, lav = pend1
                    for b in range(4):
                        epilogue1(ls, b, lps, lav)
            pend1 = (s, ps, av)

        # qw reload for layer 2 (in-place; WAR-gated on super-7 reads)
        for pidx in range(16):
            load_pair(qw2_d, pidx)

        # layer-2 correction matrix into xt_sb's space (xt now dead)
        nc.gpsimd.dma_start(c_ap(0, EC, 0, D), c_d[:])
        nc.gpsimd.dma_start(c_ap(64, 64 + EC, 0, D), c_d[:])

        # super-7's epilogue: the e2 matmuls for its ar tiles depend on it
        ls, lps, lav = pend1
        for b in range(4):
            epilogue1(ls, b, lps, lav)

        # ================= layer 2 =================
        # epilogues deferred past the next super's first chains; output
        # written via 2-bank [128 x 1024] tiles (2KB DRAM rows); last super
        # drains inline with per-bank tiles + scalar pre-cast of banks 2-3
        sc_l2 = {0: load_sc_all(2, 0)}
        pend2 = None
        for s in range(NSUP):
            scs = sc_l2[s]
            av = load_av(2, s)
            ps = [psp.tile([128, t], F32, tag="ps", name=f"ps_2_{s}_{b}")
                  for b in range(4)]
            for pidx in range(15):
                if pidx == 8 and s + 1 < NSUP:
                    sc_l2[s + 1] = load_sc_all(2, s + 1)
                chain(2, s, pidx, scs, ps, ar_v)
                if pidx == 1 and pend2 is not None:
                    ls, lps, lav = pend2
                    for half in range(2):
                        ot = outp.tile([128, 2, t], BF16, tag="ot",
                                       name=f"ot_{ls}_{half}")
                        for oti in range(2):
                            epilogue2(ls, 2 * half + oti, lps, lav, ot, oti)
                        c0 = (4 * ls + 2 * half) * t
                        nc.sync.dma_start(
                            out_d[:, c0:c0 + 2 * t].rearrange(
                                "p (i c) -> p i c", c=t), ot[:])
                    pend2 = None
            # corrections before the final pair: PSUM accumulation is
            # order-independent, so the tail drains without extra matmuls
            for b in range(4):
                c_mm(s, b, ps, r2s)
            chain(2, s, 15, scs, ps, ar_v, stop_last=True)
            if s < NSUP - 1:
                pend2 = (s, ps, av)
            else:
                # drain: scalar pre-casts banks 2-3 off the vector engine
                pre = {}
                for b in (2, 3):
                    pre[b] = yp.tile([128, t], BF16, tag="y", name=f"y2b_{b}")
                    nc.scalar.copy(pre[b][:], ps[b][:])
                for b in range(4):
                    ot = otdp.tile([128, 1, t], BF16, tag="otd",
                                   name=f"otd_{b}")
                    epilogue2(s, b, ps, av, ot, 0, pre=pre.get(b))
                    c0 = (4 * s + b) * t
                    nc.sync.dma_start(out_d[:, c0:c0 + t], ot[:, 0, :])
    nc.compile()
    return nc


def host_prep(inp, qweight, woq_scales, woq_qzeros, woq_bias, add1, add2,
              t=T_CORE, n_cores=N_CORES):
    pi = make_pi()
    kp = k_perm()
    rows1 = l1_qw_rows()
    rows2 = l2_qw_rows()
    x = inp.reshape(-1, D)
    a1 = add1.reshape(-1, D)
    a12 = (a1 + add2.reshape(-1, D))

    shifts = (np.arange(8, dtype=np.int32) * 4)
    z = ((woq_qzeros[:, :, None] >> shifts) & 0xF).reshape(G_N, D).astype(np.float32)
    zs = z * woq_scales
    c_mat = np.empty((EC, D), dtype=np.float32)
    c_mat[:G_N] = -zs[:, pi]
    c_mat[G_N] = woq_bias[pi]

    s_bf = woq_scales.astype(BF)
    pi_cols = pi.reshape(NSUP, SW)
    g1_row = 8 * np.arange(4)[:, None] + np.arange(128)[None, :] // 16
    # [s, v, p, c] -> partition-major [s, p, v, c] -> [NSUP*128, 4*SW]
    s1b = s_bf[g1_row[None, :, :, None], pi_cols[:, None, None, :]]
    s1b = np.ascontiguousarray(
        s1b.transpose(0, 2, 1, 3).reshape(NSUP * 128, 4 * SW))
    hbi = np.arange(8)
    G0 = 16 * (hbi // 4) + 4 * (hbi % 4)
    g2_row = G0[:, None] + np.arange(128)[None, :] // 32
    s2b = s_bf[g2_row[None, :, :, None], pi_cols[:, None, None, :]]
    s2b = np.ascontiguousarray(
        s2b.transpose(0, 2, 1, 3).reshape(NSUP * 128, 8 * SW))
    # host-side r2: group-summed dequantized weights (device-faithful bf16
    # rounding per element; sum-before-ar-rounding approximation is ~0.4%
    # of the correction term)
    qn = ((qweight[:, :, None] >> shifts) & 0xF).reshape(D, D)
    s_exp = s_bf.astype(np.float32)[np.arange(D) // GS]
    wb = (qn * s_exp).astype(BF).astype(np.float32)
    jpos = np.arange(D)
    g2j = jpos // 128
    hbj = 4 * ((g2j // 4) % 2) + (g2j % 4)
    ej = 16 * (hbj // 4) + 4 * (hbj % 4) + (jpos % 128) // 32
    onehot = np.zeros((D, G_N), dtype=np.float32)
    onehot[jpos, ej] = 1
    ws = wb[:, pi] @ onehot  # [D(k), 32]

    qw1 = np.ascontiguousarray(qweight[rows1])
    qw2 = np.ascontiguousarray(qweight[rows2])

    def pm(arr):
        # [D(=32 blocks of 128), t] row-major -> partition-major
        # [128, 32*t] with blocks along columns
        return np.ascontiguousarray(
            arr.reshape(32, 128, t).transpose(1, 0, 2).reshape(128, 32 * t))

    def pm_av(arr):
        # [D, t] -> per-super partition-major [NSUP*128, 4*t]
        return np.ascontiguousarray(
            arr.reshape(NSUP, 4, 128, t).transpose(0, 2, 1, 3).reshape(
                NSUP * 128, 4 * t))

    in_maps = []
    for i in range(n_cores):
        sl = slice(i * t, (i + 1) * t)
        xtb_nat = np.ascontiguousarray(x[sl].T).astype(BF)
        r1 = np.ones((EC, t), dtype=np.float32)
        r1[:G_N] = xtb_nat.astype(np.float32).reshape(G_N, GS, t).sum(axis=1)
        corr = c_mat.T @ r1  # [D(pi-order), t] layer-1 correction, exact
        a12t = (a12[sl][:, pi].T + corr).astype(BF)
        r2 = (xtb_nat.astype(np.float32).T @ ws).T \
            + onehot.T @ a12t.astype(np.float32)  # [32, t]
        r2b = np.ones((EC * 2, t), dtype=np.float32)
        r2b[0:G_N] = r2
        r2b[EC:EC + G_N] = r2
        xts = xtb_nat[kp].reshape(32, 128, t)[SLOT_INV]  # slot-major tiles
        in_maps.append({
            "qw1": qw1,
            "qw2": qw2,
            "xt_bf": np.ascontiguousarray(
                xts.transpose(1, 0, 2).reshape(128, 32 * t)),
            "s1b": s1b,
            "s2b": s2b,
            "c_mat": c_mat,
            "r2b": r2b.astype(BF),
            "a12t": pm_av(a12t),
            "a1t": pm_av(np.ascontiguousarray(a1[sl][:, pi].T).astype(BF)),
        })
    return in_maps, pi


# tile that sits at slot j of the pair-major order
SLOT_INV = np.argsort(SLOT)

_CACHE = {}


def kernel(inp, qweight, woq_scales, woq_qzeros, woq_bias, add1, add2,
           group_size=GS, _trace=False, _repeat=1):
    from concourse import bass_utils
    inp = np.asarray(inp, dtype=np.float32)
    qweight = np.asarray(qweight, dtype=np.int32)
    woq_scales = np.asarray(woq_scales, dtype=np.float32)
    woq_qzeros = np.asarray(woq_qzeros, dtype=np.int32)
    woq_bias = np.asarray(woq_bias, dtype=np.float32)
    add1 = np.asarray(add1, dtype=np.float32)
    add2 = np.asarray(add2, dtype=np.float32)

    if "nc" not in _CACHE:
        _CACHE["nc"] = build_program()
    nc = _CACHE["nc"]
    in_maps, pi = host_prep(inp, qweight, woq_scales, woq_qzeros, woq_bias,
                            add1, add2)
    import time as _time
    times = []
    res = None
    for _ in range(max(1, _repeat)):
        t0 = _time.time()
        res = bass_utils.run_bass_kernel_spmd(
            nc, in_maps, list(range(N_CORES)), trace=_trace)
        times.append(_time.time() - t0)
    _CACHE["times"] = times
    out = np.empty((N_CORES * T_CORE, D), dtype=np.float32)
    for i in range(N_CORES):
        outt = res.results[i]["outt"].astype(np.float32)
        # [128, 32*t] partition-major, blocks in g2 order -> [t, D(pi)]
        feat = outt.reshape(128, 32, T_CORE).transpose(1, 0, 2).reshape(
            D, T_CORE)
        out[i * T_CORE:(i + 1) * T_CORE][:, pi] = feat.T
    _CACHE["last_result"] = res
    return out.reshape(inp.shape[0], inp.shape[1], D)
